# revision 26
# baseline (speedup 1.0000x reference)
"""Block-local self-attention (BLOCK=128, 3-block sliding window + global token 0)
for Trainium2, sharded over 8 NeuronCores by (batch*head).

Full shapes: q/k/v (2, 16, 4096, 64) fp32, mask (2, 1, 1, 4096) fp32 (zeros).
Core c handles 4 consecutive (n*16+h) heads, as 2 "head pairs".

Design (~66-68us HW exec vs 234us baseline):
  - The host does all input marshalling: fp32->bf16 cast, K/Q transposed to
    (d, t) with each head's 64 d-rows DUPLICATED into both SBUF partition
    halves, V swizzled to (key%128, block, d) with a baked ones-column. So
    every device DMA is a fat contiguous transfer (4-8KB descriptors, ~1.3k
    total) -- no on-device casts or transposes. With 256B-row strided loads
    the SDMA queues are descriptor-rate-bound (~14ns/desc, was ~66k descs).
  - All input loads go on ONE HWDGE ring in priority order (ring FIFO =
    bandwidth priority): the SDMA engines round-robin BETWEEN rings, so a
    second ring would steal bandwidth from the critical first tile.
  - A dummy-matmul warm-up burst runs while the loads land: the PE's HAM
    clock-gate only reaches 2.4 GHz after ~4us of sustained activity, and a
    single multi-us PE gap mid-kernel re-throttles it for most of the
    kernel (this warm-up alone is worth ~1.9x).
  - The device computes only the *unnormalized* block-local attention in
    ctx^T layout plus the softmax denominator (ones-column trick); the host
    folds in the global-token term, normalizes, transposes back to (t, d),
    and patches query row 0 (full-sequence attention, computed on host).
  - Per (head, 512-query window) job, software-pipelined with a 4-job lag
    so the PE never waits on exp: scores S^T pieces (key-partition layout,
    <=384 cols, one matmul per key block j) packed into a (128, 1536) PSUM
    tile, alternating pieces between the two PE row-group halves
    (tile_position row-tiling; the duplicated K/Q layout makes adjacent
    score matmuls run concurrently in disjoint array quadrants); exp on
    ScalarE (scale=1/8 folded into the activation affine) -> P bf16; PV
    with V_j stationary accumulating ctx~ (65, 512) PSUM (row 64 =
    denominator); DVE copy to per-head (65, 4096) fp32 staging; quartered
    per-head output DMAs of ctx^T + den.
  - Steady state: ScalarE exp (~45us) and PE (~42us) are co-bottlenecks;
    PSUM's 8 banks (2x3-bank score tiles + 2x1-bank ctx tiles) pin the
    window/exp batch size.
"""

import itertools
import math

import numpy as np
import ml_dtypes

N_, H, T, D = 2, 16, 4096, 64
B = 128
NB = T // B            # 32 key/query blocks
HPC = 4                # heads per core
NCORES = 8
WQ = 512               # queries per window
NWIN = T // WQ         # 8 windows per head
SCALE = 1.0 / math.sqrt(D)
BANK = 512             # fp32 elements per PSUM bank (per partition)


def _window_pieces(w):
    """Pieces for window w: (j, qb_lo, qb_hi, N) with q blocks in window units."""
    qb0, qb1 = 4 * w, 4 * w + 3
    out = []
    for j in range(max(0, qb0 - 1), min(NB - 1, qb1 + 1) + 1):
        qlo = max(qb0, j - 1)
        qhi = min(qb1, j + 1)
        out.append((j, qlo, qhi, (qhi - qlo + 1) * B))
    return out


def _pack_offsets(sizes):
    """Pack piece sizes contiguously from 0 s.t. no piece crosses a 512-elem
    PSUM bank boundary. Returns list of offsets (same order as sizes)."""
    n = len(sizes)
    for perm in itertools.permutations(range(n)):
        off = 0
        offs = [0] * n
        ok = True
        for i in perm:
            sz = sizes[i]
            if off // BANK != (off + sz - 1) // BANK:
                ok = False
                break
            offs[i] = off
            off += sz
        if ok:
            return offs
    raise ValueError(f"cannot pack {sizes}")


_NC_CACHE = {}


def _build_nc():
    if "nc" in _NC_CACHE:
        return _NC_CACHE["nc"]

    import concourse.bacc as bacc
    import concourse.mybir as mybir
    import concourse.tile as tile

    dt = mybir.dt
    F32, BF16 = dt.float32, dt.bfloat16

    nc = bacc.Bacc("TRN2", target_bir_lowering=False, debug=False)
    # host-marshalled inputs: kq (head, dup-packed d, k/q, block, t%B) bf16
    # transposed; vs (head, key%128, block, d+ones) bf16 block-swizzled
    kq_d = nc.dram_tensor(
        "kq", [HPC, 128, 2, NB, B], BF16, kind="ExternalInput")
    vs_d = nc.dram_tensor("vs", [HPC, 128, NB, D + 1], BF16, kind="ExternalInput")
    o_d = nc.dram_tensor("o", [HPC, D + 1, T], F32, kind="ExternalOutput")

    with tile.TileContext(nc) as tc:
        with (
            tc.tile_pool(name="persist", bufs=1) as persist,
            tc.tile_pool(name="pp", bufs=6) as pp,
            tc.tile_pool(name="spsum", bufs=2, space="PSUM") as spsum,
            tc.tile_pool(name="cpsum", bufs=2, space="PSUM") as cpsum,
        ):
            HB = NB // 2  # blocks per kq half-tile
            kqh = [[persist.tile([128, 2, HB, B], BF16, tag=f"kq{h}g{g}",
                                 name=f"kq{h}g{g}")
                    for g in range(2)] for h in range(HPC)]
            vtp = [persist.tile([128, NB, D + 1], BF16, tag=f"vtp{h}",
                                name=f"vtp{h}")
                   for h in range(HPC)]
            stage = [persist.tile([D + 1, T], F32, tag=f"stage{h}", name=f"stage{h}")
                     for h in range(HPC)]

            # all loads on one HWDGE ring: ring FIFO = priority order, so
            # the first head's first K/Q half gets full bandwidth and lands
            # right as the warm-up burst ends.
            def load_kq(h, g):
                nc.sync.dma_start(
                    out=kqh[h][g][:, :, :, :],
                    in_=kq_d.ap()[h, :, :, g * HB : (g + 1) * HB, :])
            def load_v(h):
                nc.sync.dma_start(out=vtp[h][:, :, :], in_=vs_d.ap()[h])
            load_kq(0, 0)
            load_v(0)
            load_kq(0, 1)
            load_kq(1, 0)
            load_v(1)
            load_kq(1, 1)
            load_kq(2, 0)
            load_v(2)
            load_kq(2, 1)
            load_kq(3, 0)
            load_v(3)
            load_kq(3, 1)

            # PE warm-up: a back-to-back dummy matmul burst while the loads
            # are in flight, so HAM un-throttles the PE clock before the
            # first real matmul (PE is otherwise idle while loads land).
            wsrc = persist.tile([128, 512], BF16, tag="wsrc", name="wsrc")
            nc.vector.memset(wsrc[:, :], 0.0)
            wps = cpsum.tile([D + 1, WQ], F32, tag="ctx", name="warmps")
            for _ in range(12):
                nc.tensor.matmul(
                    out=wps[:, :],
                    lhsT=wsrc[:, 0 : D + 1],
                    rhs=wsrc[:, :],
                    start=True,
                    stop=True,
                )

            # compute, software-pipelined with a LAG-job lag: at step `it`
            # emit scores+exp for job it and PV+copy for job it-LAG, so exp
            # has several full PE job-periods to complete and the PE stream
            # never waits (HAM stays at full clock).
            jobs = [(h, w) for h in range(HPC) for w in range(NWIN)]
            LAG = 5
            state = {}
            for it in range(len(jobs) + LAG):
                if it < len(jobs):
                    h, w = jobs[it]
                    pieces = _window_pieces(w)
                    offs = _pack_offsets([p[3] for p in pieces])
                    tot = sum(p[3] for p in pieces)
                    sc = spsum.tile([128, 3 * BANK], F32, tag="sc")
                    # alternate pieces between the two PE row-group halves
                    # (the head's K^T/Q^T is duplicated in both partition
                    # halves) so adjacent score matmuls run concurrently in
                    # disjoint quadrants of the systolic array.
                    for i, ((j, qlo, qhi, n), off) in enumerate(zip(pieces, offs)):
                        dlo = (i % 2) * 64
                        nc.tensor.matmul(
                            out=sc[:, off : off + n],
                            lhsT=kqh[h][j // HB][dlo : dlo + 64, 0, j % HB, :],
                            rhs=kqh[h][qlo // HB][
                                dlo : dlo + 64, 1, qlo % HB : qhi % HB + 1, :
                            ],
                            start=True,
                            stop=True,
                        )
                    P = pp.tile([128, 3 * BANK], BF16, tag="p")
                    nc.scalar.activation(
                        out=P[:, 0:tot],
                        in_=sc[:, 0:tot],
                        func=mybir.ActivationFunctionType.Exp,
                        scale=SCALE,
                    )
                    state[it] = (h, w, pieces, offs, P)
                if 0 <= it - LAG < len(jobs):
                    h, w, pieces, offs, P = state.pop(it - LAG)
                    ctx = cpsum.tile([D + 1, WQ], F32, tag="ctx")
                    for i, ((j, qlo, qhi, n), off) in enumerate(zip(pieces, offs)):
                        nc.tensor.matmul(
                            out=ctx[:, (qlo - 4 * w) * B : (qhi + 1 - 4 * w) * B],
                            lhsT=vtp[h][:, j, :],
                            rhs=P[:, off : off + n],
                            start=(i == 0),
                            stop=(i == len(pieces) - 1),
                        )
                    nc.vector.tensor_copy(
                        out=stage[h][:, w * WQ : (w + 1) * WQ], in_=ctx[:, :]
                    )
                    if w % 2 == 1:
                        q0, q1 = (w - 1) * WQ, (w + 1) * WQ
                        nc.scalar.dma_start(
                            out=o_d.ap()[h, :, q0:q1],
                            in_=stage[h][:, q0:q1])

    nc.compile()
    _NC_CACHE["nc"] = nc
    return nc


def _host_globals(q, k, v):
    """Host-side tiny pieces: pg = exp(scale * K0 . Q) (zeroed for the first
    two query blocks, where token 0 is already inside the local window), and
    o0 = full-sequence attention output for query 0 (token 0 masked out, as
    the reference does via attention_mask[..., 0])."""
    k0 = k[:, :, 0, :]  # (n, h, d)
    sg = np.einsum("nhd,nhtd->nht", k0, q) * SCALE
    pg = np.exp(sg)
    pg[:, :, : 2 * B] = 0.0

    q0 = q[:, :, 0, :]  # (n, h, d)
    s0 = np.einsum("nhd,nhtd->nht", q0, k) * SCALE
    s0[:, :, 0] = -np.inf
    s0 -= s0.max(axis=-1, keepdims=True)
    p0 = np.exp(s0)
    p0 /= p0.sum(axis=-1, keepdims=True)
    o0 = np.einsum("nht,nhtd->nhd", p0, v)
    return pg, o0


def kernel(query_layer, key_layer, value_layer, attention_mask):
    from concourse.bass_utils import run_bass_kernel_spmd

    n, h, t, d = query_layer.shape
    assert (n, h, t, d) == (N_, H, T, D)

    q = np.ascontiguousarray(np.asarray(query_layer, np.float32))
    k = np.ascontiguousarray(np.asarray(key_layer, np.float32))
    v = np.ascontiguousarray(np.asarray(value_layer, np.float32))
    pg, o0 = _host_globals(q, k, v)

    nh = n * h
    bf = ml_dtypes.bfloat16
    # (nh, d, t) bf16 transposed q/k, d-rows duplicated into both halves
    kT = k.reshape(nh, T, D).transpose(0, 2, 1).astype(bf)  # (nh, D, T)
    qT = q.reshape(nh, T, D).transpose(0, 2, 1).astype(bf)
    kqs = np.stack([kT, qT], axis=2)  # (nh, D, 2, T)
    kqs = kqs.reshape(nh, 1, D, 2, NB, B)
    kq = np.ascontiguousarray(
        np.broadcast_to(kqs, (nh, 2, D, 2, NB, B))
        .reshape(nh, 128, 2, NB, B))
    # (nh, key%128, block, d+1) bf16 with ones column baked in
    vsw = np.empty((nh, B, NB, D + 1), bf)
    vsw[..., :D] = v.reshape(nh, NB, B, D).transpose(0, 2, 1, 3).astype(bf)
    vsw[..., D] = bf(1.0)

    in_maps = []
    for c in range(NCORES):
        s = slice(HPC * c, HPC * (c + 1))
        in_maps.append(
            {
                "kq": np.ascontiguousarray(kq[s]),
                "vs": np.ascontiguousarray(vsw[s]),
            }
        )

    nc = _build_nc()
    res = run_bass_kernel_spmd(nc, in_maps, core_ids=list(range(NCORES)))
    _NC_CACHE["last_result"] = res
    raw = np.concatenate([r["o"] for r in res.results], axis=0)  # (nh, 65, T)
    ctxT = raw[:, 0:D, :].reshape(n, h, D, T)
    den = raw[:, D, :].reshape(n, h, T)

    # host: global-token fold + normalize + transpose to (t, d)
    v0 = v[:, :, 0, :]  # (n, h, d)
    num = ctxT + v0[:, :, :, None] * pg[:, :, None, :]  # (n, h, d, t)
    out = (num / (den + pg)[:, :, None, :]).transpose(0, 1, 3, 2)
    out = np.ascontiguousarray(out, np.float32)
    out[:, :, 0, :] = o0
    return out


# revision 27
# speedup vs baseline: 1.0071x; 1.0071x over previous
"""Block-local self-attention (BLOCK=128, 3-block sliding window + global token 0)
for Trainium2, sharded over 8 NeuronCores by (batch*head).

Full shapes: q/k/v (2, 16, 4096, 64) fp32, mask (2, 1, 1, 4096) fp32 (zeros).
Core c handles 4 consecutive (n*16+h) heads, as 2 "head pairs".

Design (~66-68us HW exec vs 234us baseline):
  - The host does all input marshalling: fp32->bf16 cast, K/Q transposed to
    (d, t) with each head's 64 d-rows DUPLICATED into both SBUF partition
    halves, V swizzled to (key%128, block, d) with a baked ones-column. So
    every device DMA is a fat contiguous transfer (4-8KB descriptors, ~1.3k
    total) -- no on-device casts or transposes. With 256B-row strided loads
    the SDMA queues are descriptor-rate-bound (~14ns/desc, was ~66k descs).
  - All input loads go on ONE HWDGE ring in priority order (ring FIFO =
    bandwidth priority): the SDMA engines round-robin BETWEEN rings, so a
    second ring would steal bandwidth from the critical first tile.
  - A dummy-matmul warm-up burst runs while the loads land: the PE's HAM
    clock-gate only reaches 2.4 GHz after ~4us of sustained activity, and a
    single multi-us PE gap mid-kernel re-throttles it for most of the
    kernel (this warm-up alone is worth ~1.9x).
  - The device computes only the *unnormalized* block-local attention in
    ctx^T layout plus the softmax denominator (ones-column trick); the host
    folds in the global-token term, normalizes, transposes back to (t, d),
    and patches query row 0 (full-sequence attention, computed on host).
  - Per (head, 512-query window) job, software-pipelined with a 4-job lag
    so the PE never waits on exp: scores S^T pieces (key-partition layout,
    <=384 cols, one matmul per key block j) packed into a (128, 1536) PSUM
    tile, alternating pieces between the two PE row-group halves
    (tile_position row-tiling; the duplicated K/Q layout makes adjacent
    score matmuls run concurrently in disjoint array quadrants); exp on
    ScalarE (scale=1/8 folded into the activation affine) -> P bf16; PV
    with V_j stationary accumulating ctx~ (65, 512) PSUM (row 64 =
    denominator); DVE copy to per-head (65, 4096) fp32 staging; quartered
    per-head output DMAs of ctx^T + den.
  - Steady state: ScalarE exp (~45us) and PE (~42us) are co-bottlenecks;
    PSUM's 8 banks (2x3-bank score tiles + 2x1-bank ctx tiles) pin the
    window/exp batch size.
"""

import itertools
import math

import numpy as np
import ml_dtypes

N_, H, T, D = 2, 16, 4096, 64
B = 128
NB = T // B            # 32 key/query blocks
HPC = 4                # heads per core
NCORES = 8
WQ = 512               # queries per window
NWIN = T // WQ         # 8 windows per head
SCALE = 1.0 / math.sqrt(D)
BANK = 512             # fp32 elements per PSUM bank (per partition)


def _window_pieces(w):
    """Pieces for window w: (j, qb_lo, qb_hi, N) with q blocks in window units."""
    qb0, qb1 = 4 * w, 4 * w + 3
    out = []
    for j in range(max(0, qb0 - 1), min(NB - 1, qb1 + 1) + 1):
        qlo = max(qb0, j - 1)
        qhi = min(qb1, j + 1)
        out.append((j, qlo, qhi, (qhi - qlo + 1) * B))
    return out


def _pack_offsets(sizes):
    """Pack piece sizes contiguously from 0 s.t. no piece crosses a 512-elem
    PSUM bank boundary. Returns list of offsets (same order as sizes)."""
    n = len(sizes)
    for perm in itertools.permutations(range(n)):
        off = 0
        offs = [0] * n
        ok = True
        for i in perm:
            sz = sizes[i]
            if off // BANK != (off + sz - 1) // BANK:
                ok = False
                break
            offs[i] = off
            off += sz
        if ok:
            return offs
    raise ValueError(f"cannot pack {sizes}")


_NC_CACHE = {}


def _build_nc():
    if "nc" in _NC_CACHE:
        return _NC_CACHE["nc"]

    import concourse.bacc as bacc
    import concourse.mybir as mybir
    import concourse.tile as tile

    dt = mybir.dt
    F32, BF16 = dt.float32, dt.bfloat16

    nc = bacc.Bacc("TRN2", target_bir_lowering=False, debug=False)
    # host-marshalled inputs: kq (head, dup-packed d, k/q, block, t%B) bf16
    # transposed; vs (head, key%128, block, d+ones) bf16 block-swizzled
    kq_d = nc.dram_tensor(
        "kq", [HPC, 128, 2, NB, B], BF16, kind="ExternalInput")
    vs_d = nc.dram_tensor("vs", [HPC, 128, NB, D + 1], BF16, kind="ExternalInput")
    o_d = nc.dram_tensor("o", [HPC, D + 1, T], F32, kind="ExternalOutput")

    with tile.TileContext(nc) as tc:
        with (
            tc.tile_pool(name="persist", bufs=1) as persist,
            tc.tile_pool(name="pp", bufs=5) as pp,
            tc.tile_pool(name="spsum", bufs=2, space="PSUM") as spsum,
            tc.tile_pool(name="cpsum", bufs=2, space="PSUM") as cpsum,
        ):
            HB = NB // 2  # blocks per kq half-tile
            kqh = [[persist.tile([128, 2, HB, B], BF16, tag=f"kq{h}g{g}",
                                 name=f"kq{h}g{g}")
                    for g in range(2)] for h in range(HPC)]
            vtp = [persist.tile([128, NB, D + 1], BF16, tag=f"vtp{h}",
                                name=f"vtp{h}")
                   for h in range(HPC)]
            stage = [persist.tile([D + 1, T], F32, tag=f"stage{h}", name=f"stage{h}")
                     for h in range(HPC)]

            # all loads on one HWDGE ring: ring FIFO = priority order, so
            # the first head's first K/Q half gets full bandwidth and lands
            # right as the warm-up burst ends.
            def load_kq(h, g):
                nc.sync.dma_start(
                    out=kqh[h][g][:, :, :, :],
                    in_=kq_d.ap()[h, :, :, g * HB : (g + 1) * HB, :])
            def load_v(h):
                nc.sync.dma_start(out=vtp[h][:, :, :], in_=vs_d.ap()[h])
            load_kq(0, 0)
            load_v(0)
            load_kq(0, 1)
            load_kq(1, 0)
            load_v(1)
            load_kq(1, 1)
            load_kq(2, 0)
            load_v(2)
            load_kq(2, 1)
            load_kq(3, 0)
            load_v(3)
            load_kq(3, 1)

            # PE warm-up: a back-to-back dummy matmul burst while the loads
            # are in flight, so HAM un-throttles the PE clock before the
            # first real matmul (PE is otherwise idle while loads land).
            wsrc = persist.tile([128, 512], BF16, tag="wsrc", name="wsrc")
            nc.vector.memset(wsrc[:, :], 0.0)
            wps = cpsum.tile([D + 1, WQ], F32, tag="ctx", name="warmps")
            for _ in range(12):
                nc.tensor.matmul(
                    out=wps[:, :],
                    lhsT=wsrc[:, 0 : D + 1],
                    rhs=wsrc[:, :],
                    start=True,
                    stop=True,
                )

            # compute, software-pipelined with a LAG-job lag: at step `it`
            # emit scores+exp for job it and PV+copy for job it-LAG, so exp
            # has several full PE job-periods to complete and the PE stream
            # never waits (HAM stays at full clock).
            jobs = [(h, w) for h in range(HPC) for w in range(NWIN)]
            LAG = 4
            state = {}
            for it in range(len(jobs) + LAG):
                if it < len(jobs):
                    h, w = jobs[it]
                    pieces = _window_pieces(w)
                    offs = _pack_offsets([p[3] for p in pieces])
                    tot = sum(p[3] for p in pieces)
                    sc = spsum.tile([128, 3 * BANK], F32, tag="sc")
                    # alternate pieces between the two PE row-group halves
                    # (the head's K^T/Q^T is duplicated in both partition
                    # halves) so adjacent score matmuls run concurrently in
                    # disjoint quadrants of the systolic array.
                    for i, ((j, qlo, qhi, n), off) in enumerate(zip(pieces, offs)):
                        dlo = (i % 2) * 64
                        nc.tensor.matmul(
                            out=sc[:, off : off + n],
                            lhsT=kqh[h][j // HB][dlo : dlo + 64, 0, j % HB, :],
                            rhs=kqh[h][qlo // HB][
                                dlo : dlo + 64, 1, qlo % HB : qhi % HB + 1, :
                            ],
                            start=True,
                            stop=True,
                        )
                    P = pp.tile([128, 3 * BANK], BF16, tag="p")
                    nc.scalar.activation(
                        out=P[:, 0:tot],
                        in_=sc[:, 0:tot],
                        func=mybir.ActivationFunctionType.Exp,
                        scale=SCALE,
                    )
                    state[it] = (h, w, pieces, offs, P)
                if 0 <= it - LAG < len(jobs):
                    h, w, pieces, offs, P = state.pop(it - LAG)
                    ctx = cpsum.tile([D + 1, WQ], F32, tag="ctx")
                    for i, ((j, qlo, qhi, n), off) in enumerate(zip(pieces, offs)):
                        nc.tensor.matmul(
                            out=ctx[:, (qlo - 4 * w) * B : (qhi + 1 - 4 * w) * B],
                            lhsT=vtp[h][:, j, :],
                            rhs=P[:, off : off + n],
                            start=(i == 0),
                            stop=(i == len(pieces) - 1),
                        )
                    nc.vector.tensor_copy(
                        out=stage[h][:, w * WQ : (w + 1) * WQ], in_=ctx[:, :]
                    )
                    if w % 2 == 1:
                        q0, q1 = (w - 1) * WQ, (w + 1) * WQ
                        nc.scalar.dma_start(
                            out=o_d.ap()[h, :, q0:q1],
                            in_=stage[h][:, q0:q1])

    nc.compile()
    _NC_CACHE["nc"] = nc
    return nc


def _host_globals(q, k, v):
    """Host-side tiny pieces: pg = exp(scale * K0 . Q) (zeroed for the first
    two query blocks, where token 0 is already inside the local window), and
    o0 = full-sequence attention output for query 0 (token 0 masked out, as
    the reference does via attention_mask[..., 0])."""
    k0 = k[:, :, 0, :]  # (n, h, d)
    sg = np.einsum("nhd,nhtd->nht", k0, q) * SCALE
    pg = np.exp(sg)
    pg[:, :, : 2 * B] = 0.0

    q0 = q[:, :, 0, :]  # (n, h, d)
    s0 = np.einsum("nhd,nhtd->nht", q0, k) * SCALE
    s0[:, :, 0] = -np.inf
    s0 -= s0.max(axis=-1, keepdims=True)
    p0 = np.exp(s0)
    p0 /= p0.sum(axis=-1, keepdims=True)
    o0 = np.einsum("nht,nhtd->nhd", p0, v)
    return pg, o0


def kernel(query_layer, key_layer, value_layer, attention_mask):
    from concourse.bass_utils import run_bass_kernel_spmd

    n, h, t, d = query_layer.shape
    assert (n, h, t, d) == (N_, H, T, D)

    q = np.ascontiguousarray(np.asarray(query_layer, np.float32))
    k = np.ascontiguousarray(np.asarray(key_layer, np.float32))
    v = np.ascontiguousarray(np.asarray(value_layer, np.float32))
    pg, o0 = _host_globals(q, k, v)

    nh = n * h
    bf = ml_dtypes.bfloat16
    # (nh, d, t) bf16 transposed q/k, d-rows duplicated into both halves
    kT = k.reshape(nh, T, D).transpose(0, 2, 1).astype(bf)  # (nh, D, T)
    qT = q.reshape(nh, T, D).transpose(0, 2, 1).astype(bf)
    kqs = np.stack([kT, qT], axis=2)  # (nh, D, 2, T)
    kqs = kqs.reshape(nh, 1, D, 2, NB, B)
    kq = np.ascontiguousarray(
        np.broadcast_to(kqs, (nh, 2, D, 2, NB, B))
        .reshape(nh, 128, 2, NB, B))
    # (nh, key%128, block, d+1) bf16 with ones column baked in
    vsw = np.empty((nh, B, NB, D + 1), bf)
    vsw[..., :D] = v.reshape(nh, NB, B, D).transpose(0, 2, 1, 3).astype(bf)
    vsw[..., D] = bf(1.0)

    in_maps = []
    for c in range(NCORES):
        s = slice(HPC * c, HPC * (c + 1))
        in_maps.append(
            {
                "kq": np.ascontiguousarray(kq[s]),
                "vs": np.ascontiguousarray(vsw[s]),
            }
        )

    nc = _build_nc()
    res = run_bass_kernel_spmd(nc, in_maps, core_ids=list(range(NCORES)))
    _NC_CACHE["last_result"] = res
    raw = np.concatenate([r["o"] for r in res.results], axis=0)  # (nh, 65, T)
    ctxT = raw[:, 0:D, :].reshape(n, h, D, T)
    den = raw[:, D, :].reshape(n, h, T)

    # host: global-token fold + normalize + transpose to (t, d)
    v0 = v[:, :, 0, :]  # (n, h, d)
    num = ctxT + v0[:, :, :, None] * pg[:, :, None, :]  # (n, h, d, t)
    out = (num / (den + pg)[:, :, None, :]).transpose(0, 1, 3, 2)
    out = np.ascontiguousarray(out, np.float32)
    out[:, :, 0, :] = o0
    return out
